# revision 2
# baseline (speedup 1.0000x reference)
"""Trainium2 Bass kernel v7 for the CNN-VAE loss:

    prob = einsum('klb,hwb->klhw', beta, A) * 5000
    mse  = mean(sum(|x - prob[:, :, None]|^2, axis=1))

Layout: (k,l) on the 128 SBUF partitions; hw sharded 8 ways (5000 px/core).

Key facts this version is built around (measured on this part):
- Chip-wide HBM read bw under 8-core SPMD is ~1.4 TB/s -> ~175 GB/s/core,
  so x MUST ship as fp8 (1.92MB/core); the gpsimd SWDGE queue casts
  fp8->bf16 in the DMA datapath (~13us stream, overlapped).
- A (cb) is split across all three DMA queues as each queue's first
  descriptor so matmuls start ~10us.
- ScalarE does the PSUM->bf16 negprob casts (ACT Copy) + a tuned share of
  Square+accum; VectorE does the broadcast adds (2x packed) + custom
  TENSOR_TENSOR_REDUCE squares; PE absorbs ~23 Gram chunks (diff^T diff
  into one PSUM bank, LDW+MM serialize at ~314ns/chunk); gpsimd adds one
  small group.
- Minimal epilogue (no tile barriers / sem clears / final drain).
"""

import numpy as np
import ml_dtypes

K, L, NB, H, W = 16, 8, 3, 200, 200
KL = K * L
C = 3
HW = H * W
N_CORES = 8
HW_SHARD = HW // N_CORES  # 5000
SCALE = 5000.0
DENOM = float(K * C * H * W)
BANK = 512

# (pixels, add_engine, act_sq_cols, pe_gram_chunks)
GROUPS = [
    (500, "v", 760, 2),
    (500, "g", 760, 2),
    (1000, "v", 1480, 5),
    (1000, "v", 1480, 5),
    (1000, "v", 1480, 5),
    (750, "v", 840, 4),
    (250, "v", 0, 0),
]
assert sum(g[0] for g in GROUPS) == HW_SHARD
NG = len(GROUPS)
HEAD = GROUPS[0][0]
CB_M = 2000
CB_T = HW_SHARD - HEAD - CB_M
CONST_W = KL + HW_SHARD
NCOL = 2 * NG + 1

_NC = None


def _build():
    global _NC
    if _NC is not None:
        return _NC
    from contextlib import ExitStack

    import concourse.bacc as bacc
    import concourse.mybir as mybir
    import concourse.tile as tile
    from concourse import dve_ops

    f32 = mybir.dt.float32
    bf16 = mybir.dt.bfloat16
    f8 = mybir.dt.float8e4
    Copy = mybir.ActivationFunctionType.Copy
    Square = mybir.ActivationFunctionType.Square

    class FastTC(tile.TileContext):
        def _drain_and_barrier(self, tick_clock, wait_clock):
            popped = self.nc._tile_sem_poison_stack.pop()
            assert popped is self._sem_poison

    nc = bacc.Bacc("TRN2", target_bir_lowering=False, debug=False)

    xs = nc.dram_tensor("xs", [KL, C * HW_SHARD], f8, kind="ExternalInput").ap()
    cbh = nc.dram_tensor("cbh", [NB, KL + HEAD], bf16, kind="ExternalInput").ap()
    cbm = nc.dram_tensor("cbm", [NB, CB_M], bf16, kind="ExternalInput").ap()
    cbt = nc.dram_tensor("cbt", [NB, CB_T], bf16, kind="ExternalInput").ap()
    aux = nc.dram_tensor("aux", [KL, 128], bf16, kind="ExternalInput").ap()
    out = nc.dram_tensor("out", [KL, NCOL], f32, kind="ExternalOutput").ap()

    with FastTC(nc) as tc, ExitStack() as ctx:
        const = ctx.enter_context(tc.tile_pool(name="const", bufs=1))
        ppool = ctx.enter_context(tc.tile_pool(name="psum", bufs=3, space="PSUM"))
        gpool = ctx.enter_context(tc.tile_pool(name="gram", bufs=1, space="PSUM"))

        cb_sb = const.tile([NB, CONST_W], bf16)
        nc.sync.dma_start(cb_sb[:, : KL + HEAD], cbh[:])
        nc.scalar.dma_start(cb_sb[:, KL + HEAD : KL + HEAD + CB_M], cbm[:])
        nc.gpsimd.dma_start(cb_sb[:, KL + HEAD + CB_M :], cbt[:])
        ident = const.tile([KL, 128], bf16, name="ident")
        nc.sync.dma_start(ident[:], aux[:])

        offs = []
        off = 0
        for sz, *_ in GROUPS:
            offs.append(off)
            off += sz

        xts = {}
        for g, (sz, *_) in enumerate(GROUPS):
            xt = const.tile([KL, C * sz], bf16, name=f"xt{g}")
            xts[g] = xt
            o = offs[g]
            nc.gpsimd.dma_start(xt[:], xs[:, C * o : C * (o + sz)])

        acc = const.tile([KL, NCOL], f32, name="acc")
        gram = gpool.tile([KL, 128], f32, name="gram")
        bts = cb_sb[:, :KL]

        def emit_ttr(g):
            sz, _, act_cols, pe_chunks = GROUPS[g]
            ncols = C * sz
            p1 = act_cols + 128 * pe_chunks
            if p1 >= ncols:
                return
            sl = xts[g][:, p1:ncols]
            nc.vector._custom_dve(
                dve_ops.TENSOR_TENSOR_REDUCE,
                out=sl,
                in0=sl,
                in1=sl,
                s0=0.0,
                s1=1.0,
                accum_out=acc[:, NG + g : NG + g + 1],
            )

        ttr_emitted = set()
        for g, (sz, addeng, act_cols, pe_chunks) in enumerate(GROUPS):
            off = offs[g]
            nb = (sz + BANK - 1) // BANK
            pp = ppool.tile([KL, nb, BANK], f32)
            with tc.high_priority():
                for h in range((sz + 499) // 500):
                    w = min(500, sz - h * 500)
                    nc.tensor.matmul(
                        pp[:, h, :w],
                        bts,
                        cb_sb[:, KL + off + h * 500 : KL + off + h * 500 + w],
                        start=True,
                        stop=True,
                    )
                pb = const.tile([KL, sz], bf16, name=f"pb{g}")
                if sz > 500 and sz % 500 == 0:
                    nc.scalar.activation(
                        pb[:].rearrange("p (u f) -> p u f", f=500),
                        pp[:, :, :500],
                        Copy,
                    )
                elif sz <= 500:
                    nc.scalar.activation(pb[:], pp[:, 0, :sz], Copy)
                else:
                    for h in range((sz + 499) // 500):
                        w = min(500, sz - h * 500)
                        nc.scalar.activation(
                            pb[:, h * 500 : h * 500 + w], pp[:, h, :w], Copy
                        )

            xt = xts[g]
            xv = xt[:].rearrange("p (c f) -> p c f", c=C)
            eng = nc.vector if addeng == "v" else nc.gpsimd
            eng.tensor_add(xv, xv, pb[:].unsqueeze(1).broadcast_to([KL, C, sz]))
            # fill the DVE's wait-for-x gaps with an older group's TTR
            if addeng == "v" and g >= 2:
                tg = g - 2
                if tg not in ttr_emitted:
                    emit_ttr(tg)
                    ttr_emitted.add(tg)

        gram_i = 0
        n_gram_total = sum(pe for *_x, pe in GROUPS)
        for g, (sz, addeng, act_cols, pe_chunks) in enumerate(GROUPS):
            dt = xts[g][:]
            if act_cols > 0:
                nc.scalar.activation(
                    dt[:, :act_cols], dt[:, :act_cols], Square,
                    accum_out=acc[:, g : g + 1],
                )
            with tc.high_priority(offset=-10000):
                for c in range(pe_chunks):
                    ch = dt[:, act_cols + 128 * c : act_cols + 128 * (c + 1)]
                    nc.tensor.matmul(
                        gram[:],
                        ch,
                        ch,
                        start=(gram_i == 0),
                        stop=(gram_i == n_gram_total - 1),
                    )
                    gram_i += 1
        for g in range(NG):
            if g not in ttr_emitted:
                emit_ttr(g)
                ttr_emitted.add(g)

        gscr = const.tile([KL, 128], bf16, name="gscr")
        nc.vector._custom_dve(
            dve_ops.TENSOR_TENSOR_REDUCE,
            out=gscr[:],
            in0=gram[:],
            in1=ident[:],
            s0=0.0,
            s1=1.0,
            accum_out=acc[:, 2 * NG : 2 * NG + 1],
        )

        nc.sync.dma_start(out[:, :NG], acc[:, :NG])
        nc.sync.dma_start(out[:, NG:], acc[:, NG:])

    nc.compile()
    _NC = nc
    return nc


def _make_in_maps(x, beta, A):
    bf16 = ml_dtypes.bfloat16
    f8 = ml_dtypes.float8_e4m3
    x = np.asarray(x, dtype=np.float32)
    beta = np.asarray(beta, dtype=np.float32)
    A = np.asarray(A, dtype=np.float32)

    xr = np.ascontiguousarray(x.reshape(KL, C, HW)).astype(f8)
    at_full = (A.reshape(HW, NB).T).astype(bf16)
    bts = (beta.reshape(KL, NB).T * -SCALE).astype(bf16)
    ident = np.eye(KL, 128, dtype=np.float32).astype(bf16)

    in_maps = []
    for i in range(N_CORES):
        lo = i * HW_SHARD
        at = at_full[:, lo : lo + HW_SHARD]
        cbh = np.ascontiguousarray(np.concatenate([bts, at[:, :HEAD]], axis=1))
        cbm = np.ascontiguousarray(at[:, HEAD : HEAD + CB_M])
        cbt = np.ascontiguousarray(at[:, HEAD + CB_M :])
        parts = []
        off = 0
        for sz, *_ in GROUPS:
            parts.append(
                xr[:, :, lo + off : lo + off + sz].reshape(KL, C * sz)
            )
            off += sz
        xcore = np.ascontiguousarray(np.concatenate(parts, axis=1))
        in_maps.append(
            {"xs": xcore, "cbh": cbh, "cbm": cbm, "cbt": cbt, "aux": ident}
        )
    return in_maps


def _run(in_maps, trace=False, **kwargs):
    from concourse import bass_utils

    nc = _build()
    return bass_utils.run_bass_kernel_spmd(
        nc, in_maps, list(range(N_CORES)), trace=trace, **kwargs
    )


def _combine(results):
    total = 0.0
    for r in results:
        o = np.asarray(r["out"], dtype=np.float64)
        total += float(o.sum())
    return np.float32(total / DENOM)


def kernel(x, beta, A):
    res = _run(_make_in_maps(x, beta, A))
    return _combine(res.results)


# revision 3
# speedup vs baseline: 1.0244x; 1.0244x over previous
"""Trainium2 Bass kernel v7 for the CNN-VAE loss:

    prob = einsum('klb,hwb->klhw', beta, A) * 5000
    mse  = mean(sum(|x - prob[:, :, None]|^2, axis=1))

Layout: (k,l) on the 128 SBUF partitions; hw sharded 8 ways (5000 px/core).

Key facts this version is built around (measured on this part):
- Chip-wide HBM read bw under 8-core SPMD is ~1.4 TB/s -> ~175 GB/s/core,
  so x MUST ship as fp8 (1.92MB/core); the gpsimd SWDGE queue casts
  fp8->bf16 in the DMA datapath (~13us stream, overlapped).
- A (cb) is split across all three DMA queues as each queue's first
  descriptor so matmuls start ~10us.
- ScalarE does the PSUM->bf16 negprob casts (ACT Copy) + a tuned share of
  Square+accum; VectorE does the broadcast adds (2x packed) + custom
  TENSOR_TENSOR_REDUCE squares; PE absorbs ~23 Gram chunks (diff^T diff
  into one PSUM bank, LDW+MM serialize at ~314ns/chunk); gpsimd adds one
  small group.
- Minimal epilogue (no tile barriers / sem clears / final drain).
"""

import numpy as np
import ml_dtypes

K, L, NB, H, W = 16, 8, 3, 200, 200
KL = K * L
C = 3
HW = H * W
N_CORES = 8
HW_SHARD = HW // N_CORES  # 5000
SCALE = 5000.0
DENOM = float(K * C * H * W)
BANK = 512

# (pixels, add_engine, act_sq_cols, pe_gram_chunks)
# all adds on the DVE: a gpsimd tensor_add stalls the SWDGE x stream (the
# gpsimd engine feeds the software DGE) and slows concurrent DVE adds
GROUPS = [
    (500, "v", 760, 2),
    (500, "v", 760, 2),
    (1000, "v", 1480, 6),
    (1000, "v", 1480, 6),
    (1000, "v", 1480, 5),
    (750, "v", 1400, 0),
    (250, "v", 400, 0),
]
assert sum(g[0] for g in GROUPS) == HW_SHARD
NG = len(GROUPS)
HEAD = GROUPS[0][0]
CB_M = 2000
CB_T = HW_SHARD - HEAD - CB_M
CONST_W = KL + HW_SHARD
NCOL = 2 * NG + 1

_NC = None


def _build():
    global _NC
    if _NC is not None:
        return _NC
    from contextlib import ExitStack

    import concourse.bacc as bacc
    import concourse.mybir as mybir
    import concourse.tile as tile
    from concourse import dve_ops

    f32 = mybir.dt.float32
    bf16 = mybir.dt.bfloat16
    f8 = mybir.dt.float8e4
    Copy = mybir.ActivationFunctionType.Copy
    Square = mybir.ActivationFunctionType.Square

    class FastTC(tile.TileContext):
        def _drain_and_barrier(self, tick_clock, wait_clock):
            popped = self.nc._tile_sem_poison_stack.pop()
            assert popped is self._sem_poison

    nc = bacc.Bacc("TRN2", target_bir_lowering=False, debug=False)

    xs = nc.dram_tensor("xs", [KL, C * HW_SHARD], f8, kind="ExternalInput").ap()
    cbh = nc.dram_tensor("cbh", [NB, KL + HEAD], bf16, kind="ExternalInput").ap()
    cbm = nc.dram_tensor("cbm", [NB, CB_M], bf16, kind="ExternalInput").ap()
    cbt = nc.dram_tensor("cbt", [NB, CB_T], bf16, kind="ExternalInput").ap()
    aux = nc.dram_tensor("aux", [KL, 128], bf16, kind="ExternalInput").ap()
    out = nc.dram_tensor("out", [KL, NCOL], f32, kind="ExternalOutput").ap()

    with FastTC(nc) as tc, ExitStack() as ctx:
        const = ctx.enter_context(tc.tile_pool(name="const", bufs=1))
        ppool = ctx.enter_context(tc.tile_pool(name="psum", bufs=3, space="PSUM"))
        gpool = ctx.enter_context(tc.tile_pool(name="gram", bufs=1, space="PSUM"))

        cb_sb = const.tile([NB, CONST_W], bf16)
        nc.sync.dma_start(cb_sb[:, : KL + HEAD], cbh[:])
        nc.scalar.dma_start(cb_sb[:, KL + HEAD : KL + HEAD + CB_M], cbm[:])
        nc.gpsimd.dma_start(cb_sb[:, KL + HEAD + CB_M :], cbt[:])
        ident = const.tile([KL, 128], bf16, name="ident")
        nc.sync.dma_start(ident[:], aux[:])

        offs = []
        off = 0
        for sz, *_ in GROUPS:
            offs.append(off)
            off += sz

        xts = {}
        for g, (sz, *_) in enumerate(GROUPS):
            xt = const.tile([KL, C * sz], bf16, name=f"xt{g}")
            xts[g] = xt
            o = offs[g]
            nc.gpsimd.dma_start(xt[:], xs[:, C * o : C * (o + sz)])

        acc = const.tile([KL, NCOL], f32, name="acc")
        gram = gpool.tile([KL, 128], f32, name="gram")
        bts = cb_sb[:, :KL]

        def emit_ttr(g):
            sz, _, act_cols, pe_chunks = GROUPS[g]
            ncols = C * sz
            p1 = act_cols + 128 * pe_chunks
            if p1 >= ncols:
                return
            sl = xts[g][:, p1:ncols]
            nc.vector._custom_dve(
                dve_ops.TENSOR_TENSOR_REDUCE,
                out=sl,
                in0=sl,
                in1=sl,
                s0=0.0,
                s1=1.0,
                accum_out=acc[:, NG + g : NG + g + 1],
            )

        ttr_emitted = set()
        for g, (sz, addeng, act_cols, pe_chunks) in enumerate(GROUPS):
            off = offs[g]
            nb = (sz + BANK - 1) // BANK
            pp = ppool.tile([KL, nb, BANK], f32)
            with tc.high_priority():
                for h in range((sz + 499) // 500):
                    w = min(500, sz - h * 500)
                    nc.tensor.matmul(
                        pp[:, h, :w],
                        bts,
                        cb_sb[:, KL + off + h * 500 : KL + off + h * 500 + w],
                        start=True,
                        stop=True,
                    )
                pb = const.tile([KL, sz], bf16, name=f"pb{g}")
                if sz > 500 and sz % 500 == 0:
                    nc.scalar.activation(
                        pb[:].rearrange("p (u f) -> p u f", f=500),
                        pp[:, :, :500],
                        Copy,
                    )
                elif sz <= 500:
                    nc.scalar.activation(pb[:], pp[:, 0, :sz], Copy)
                else:
                    for h in range((sz + 499) // 500):
                        w = min(500, sz - h * 500)
                        nc.scalar.activation(
                            pb[:, h * 500 : h * 500 + w], pp[:, h, :w], Copy
                        )

            xt = xts[g]
            xv = xt[:].rearrange("p (c f) -> p c f", c=C)
            eng = nc.vector if addeng == "v" else nc.gpsimd
            eng.tensor_add(xv, xv, pb[:].unsqueeze(1).broadcast_to([KL, C, sz]))
            # fill the DVE's wait-for-x gaps with an older group's TTR
            if addeng == "v" and g >= 2:
                tg = g - 2
                if tg not in ttr_emitted:
                    emit_ttr(tg)
                    ttr_emitted.add(tg)

        gram_i = 0
        n_gram_total = sum(pe for *_x, pe in GROUPS)
        for g, (sz, addeng, act_cols, pe_chunks) in enumerate(GROUPS):
            dt = xts[g][:]
            if act_cols > 0:
                nc.scalar.activation(
                    dt[:, :act_cols], dt[:, :act_cols], Square,
                    accum_out=acc[:, g : g + 1],
                )
            with tc.high_priority(offset=-10000):
                for c in range(pe_chunks):
                    ch = dt[:, act_cols + 128 * c : act_cols + 128 * (c + 1)]
                    nc.tensor.matmul(
                        gram[:],
                        ch,
                        ch,
                        start=(gram_i == 0),
                        stop=(gram_i == n_gram_total - 1),
                    )
                    gram_i += 1
        for g in range(NG):
            if g not in ttr_emitted:
                emit_ttr(g)
                ttr_emitted.add(g)

        gscr = const.tile([KL, 128], bf16, name="gscr")
        nc.vector._custom_dve(
            dve_ops.TENSOR_TENSOR_REDUCE,
            out=gscr[:],
            in0=gram[:],
            in1=ident[:],
            s0=0.0,
            s1=1.0,
            accum_out=acc[:, 2 * NG : 2 * NG + 1],
        )

        nc.sync.dma_start(out[:, :NG], acc[:, :NG])
        nc.sync.dma_start(out[:, NG:], acc[:, NG:])

    nc.compile()
    _NC = nc
    return nc


def _make_in_maps(x, beta, A):
    bf16 = ml_dtypes.bfloat16
    f8 = ml_dtypes.float8_e4m3
    x = np.asarray(x, dtype=np.float32)
    beta = np.asarray(beta, dtype=np.float32)
    A = np.asarray(A, dtype=np.float32)

    xr = np.ascontiguousarray(x.reshape(KL, C, HW)).astype(f8)
    at_full = (A.reshape(HW, NB).T).astype(bf16)
    bts = (beta.reshape(KL, NB).T * -SCALE).astype(bf16)
    ident = np.eye(KL, 128, dtype=np.float32).astype(bf16)

    in_maps = []
    for i in range(N_CORES):
        lo = i * HW_SHARD
        at = at_full[:, lo : lo + HW_SHARD]
        cbh = np.ascontiguousarray(np.concatenate([bts, at[:, :HEAD]], axis=1))
        cbm = np.ascontiguousarray(at[:, HEAD : HEAD + CB_M])
        cbt = np.ascontiguousarray(at[:, HEAD + CB_M :])
        parts = []
        off = 0
        for sz, *_ in GROUPS:
            parts.append(
                xr[:, :, lo + off : lo + off + sz].reshape(KL, C * sz)
            )
            off += sz
        xcore = np.ascontiguousarray(np.concatenate(parts, axis=1))
        in_maps.append(
            {"xs": xcore, "cbh": cbh, "cbm": cbm, "cbt": cbt, "aux": ident}
        )
    return in_maps


def _run(in_maps, trace=False, **kwargs):
    from concourse import bass_utils

    nc = _build()
    return bass_utils.run_bass_kernel_spmd(
        nc, in_maps, list(range(N_CORES)), trace=trace, **kwargs
    )


def _combine(results):
    total = 0.0
    for r in results:
        o = np.asarray(r["out"], dtype=np.float64)
        total += float(o.sum())
    return np.float32(total / DENOM)


def kernel(x, beta, A):
    res = _run(_make_in_maps(x, beta, A))
    return _combine(res.results)
